# revision 14
# baseline (speedup 1.0000x reference)
"""Trainium2 Bass kernel for nn_AtLocPlusCriterion_VO.

loss = exp(-srx)*mean|vo_t - tg_t| + srx + exp(-srq)*mean|vo_q - tg_q| + srq
with vo = calc_vo_logq(pred[:-1], pred[1:]) (relative SE(3) pose, log-quaternion).

Sequence-parallel across 8 NeuronCores (1-row halo per shard). Inputs are
resharded host-side into component-major (SoA) bf16 planes so every on-device
vector op runs in the DVE 2x bf16 mode. Per core: 1956 pairs per SBUF
partition, 3 tiles of 652. Row phase (qexp via quarter-angle Sin LUT + Ln/Exp
roots) feeds a pair phase (rotation by two cross products, quaternion product,
log map via arctan) on VectorE; unary work (squares, LUTs, |x| + accumulate)
runs on ScalarE. Emission is phase-looped across tiles so scalar LUT work
groups into 4 activation-table loads (row Ln/Exp x3 tiles, row Sin x3,
pair Ln/Exp x3, pair Arctan x3). Mean-L1 reduces through per-partition
accumulators; host sums 8x[128,2].
"""
import os
import numpy as np
import ml_dtypes

N_CORES = 8
T_FULL = 2_000_000
NPAIRS = T_FULL - 1          # 1_999_999
D = 1956                     # pairs per partition per core
C = 652                      # pairs per tile (3 tiles)
NT = 3
R = C + 1
PPC = 128 * D                # 250_368 pairs per core
PAIRS_PAD = N_CORES * PPC    # 2_002_944
ROWS_PAD = PAIRS_PAD + 1

PRED_LEN = 6 * (PPC + 1)
TARG_LEN = 6 * PPC

LN4 = float(np.log(4.0))
LN4SQ2 = float(np.log(4.0 * np.sqrt(2.0)))   # i2n carries 4*sqrt2
PI2 = float(np.pi / 2.0)
SQ2 = float(np.sqrt(2.0))

_BUILT = {}


def _patch_act_tables():
    import concourse.bacc as bacc_mod
    import concourse.hw_specs as hw

    if getattr(bacc_mod, "_vo_tables_patched", False):
        return
    orig = hw.get_activation_tables

    def steered(arch, _orig=orig):
        from concourse import mybir as _mb
        AF = _mb.ActivationFunctionType
        t = {k: set(v) for k, v in _orig(arch).items()}
        # Keep all 24 entries (act_func_set_id indexes the original list);
        # drop ln/exp/arctan from the earlier sets so the table-load pass
        # resolves them to natural_log_exp_and_others / trig_and_small.
        t.get("natural_log", set()).discard(AF.Ln)
        t.get("exp_and_others", set()).discard(AF.Exp)
        t.get("sigmoid_and_others", set()).discard(AF.Arctan)
        return t

    bacc_mod.get_activation_tables = steered
    bacc_mod._vo_tables_patched = True


def _build():
    from concourse import bacc, tile, mybir
    from concourse.ap import AP
    from concourse.bass import _add_dep_helper

    _patch_act_tables()

    f32, bf16 = mybir.dt.float32, mybir.dt.bfloat16
    OP = mybir.AluOpType
    AF = mybir.ActivationFunctionType

    nc = bacc.Bacc("TRN2", target_bir_lowering=False, debug=False,
                   num_devices=N_CORES)
    pred_h = nc.declare_dram_parameter("pred", [PRED_LEN], bf16, isOutput=False)
    targ_h = nc.declare_dram_parameter("targ", [TARG_LEN], bf16, isOutput=False)
    out_h = nc.declare_dram_parameter("out", [128, 2], f32, isOutput=True)

    for v in (1e-16, -LN4, LN4SQ2, PI2):
        v = float(v)
        if (f32, v) not in nc.const_aps.aps:
            t = nc.alloc_sbuf_tensor(f"uconst-{v}", [128, 1], f32)
            nc.gpsimd.memset(t.ap(), v)
            nc.const_aps.aps[(f32, v)] = t.ap()
    nc.all_engine_barrier()

    PL_P = PPC + 1   # pred plane length
    PL_T = PPC       # targ plane length

    def sb(tile_, off, dims):
        base = tile_[:, :]
        return AP(base.tensor, base.offset + off,
                  [[base.ap.to_list()[0][0], 128]] + dims)

    acc_ts, acc_qs = [], []
    groups = {}  # (tile, name) -> list of act instructions

    with tile.TileContext(nc) as tc:

        def mkact(tile_i, group, *args, **kw):
            ins = nc.scalar.activation(*args, **kw)
            if group is not None:
                groups.setdefault((tile_i, group), []).append(ins)
            return ins

        with (
            tc.tile_pool(name="inp", bufs=1) as pin,
            tc.tile_pool(name="rowp", bufs=1) as prow,
            tc.tile_pool(name="scr", bufs=1) as pscr,
            tc.tile_pool(name="accp", bufs=8) as pacc,
        ):
            state = {}

            def dma_phase(t):
                # ---- DMA: component-major bf16 planes, all contiguous ----
                tv = pin.tile([128, 3 * R], bf16, tag=f"tv{t}")   # logq comps
                nc.sync.dma_start(
                    tv[:].rearrange("p (c r) -> p c r", c=3),
                    AP(pred_h, 3 * PL_P + t * C, [[D, 128], [PL_P, 3], [1, R]]))
                tt = pin.tile([128, 3 * R], bf16, tag=f"tt{t % 2}")   # t comps
                nc.sync.dma_start(
                    tt[:].rearrange("p (c r) -> p c r", c=3),
                    AP(pred_h, t * C, [[D, 128], [PL_P, 3], [1, R]]))
                gtt = pin.tile([128, 3 * C], bf16, tag=f"gtt{t}")  # targ t
                nc.sync.dma_start(
                    gtt[:].rearrange("p (c r) -> p c r", c=3),
                    AP(targ_h, t * C, [[D, 128], [PL_T, 3], [1, C]]))
                gtq = pin.tile([128, 3 * C], bf16, tag=f"gtq{t}")  # targ q
                nc.sync.dma_start(
                    gtq[:].rearrange("p (c r) -> p c r", c=3),
                    AP(targ_h, 3 * PL_T + t * C, [[D, 128], [PL_T, 3], [1, C]]))
                state[t] = {"tv": tv, "tt": tt, "gtt": gtt, "gtq": gtq}

            def row_n2(t):
                st = state[t]
                sq = pscr.tile([128, 3 * R], bf16, tag="sq")
                mkact(t, None, sq[:], st["tv"][:], AF.Square)
                n2a = pscr.tile([128, R], bf16, tag="n2a")
                nc.vector.tensor_tensor(n2a[:], sq[:, 0:R], sq[:, R:2 * R], OP.add)
                n2 = pscr.tile([128, R], bf16, tag=f"n2{t}")
                nc.vector.tensor_tensor(n2[:], n2a[:], sq[:, 2 * R:3 * R], OP.add)
                st["n2"] = n2

            def early_g1(t):
                # translation deltas: independent of the row LUT chain, runs
                # on VectorE while ScalarE does table loads + Ln/Exp/Sin.
                st = state[t]
                tt = st["tt"]
                g1 = pscr.tile([128, 5 * C], bf16, tag=f"g1{t % 2}")
                nc.vector.tensor_tensor(
                    sb(g1, 0, [[C, 3], [1, C]]),
                    sb(tt, 1, [[R, 3], [1, C]]),
                    sb(tt, 0, [[R, 3], [1, C]]), OP.subtract)
                nc.vector.tensor_copy(g1[:, 3 * C:5 * C], g1[:, 0:2 * C])
                st["g1"] = g1

            def row_lut(t):
                st = state[t]
                l = pscr.tile([128, R], f32, tag="l")
                mkact(t, 'rowLE', l[:], st["n2"][:], AF.Ln, bias=1e-16)
                n4 = pscr.tile([128, R], bf16, tag=f"n4{t}")
                mkact(t, 'rowLE', n4[:], l[:], AF.Exp, bias=-LN4, scale=0.5)
                i2n = pscr.tile([128, R], bf16, tag=f"i2n{t}")
                mkact(t, 'rowLE', i2n[:], l[:], AF.Exp, bias=LN4SQ2, scale=-0.5)
                st["n4"], st["i2n"] = n4, i2n

            def row_trig(t):
                st = state[t]
                s4 = pscr.tile([128, R], bf16, tag=f"s4{t}")
                mkact(t, 'rowTR', s4[:], st["n4"][:], AF.Sin)
                c4 = pscr.tile([128, R], bf16, tag=f"c4{t}")
                mkact(t, 'rowTR', c4[:], st["n4"][:], AF.Sin, bias=PI2)
                st["s4"], st["c4"] = s4, c4

            def row_fin(t):
                st = state[t]
                sc = pscr.tile([128, R], bf16, tag="scx")
                nc.vector.tensor_tensor(sc[:], st["s4"][:], st["c4"][:], OP.mult)
                s4sq = pscr.tile([128, R], bf16, tag="s4sq")
                mkact(t, None, s4sq[:], st["s4"][:], AF.Square)
                cs = pscr.tile([128, R], bf16, tag="cs")
                mkact(t, None, cs[:], s4sq[:], AF.Copy, bias=1.0, scale=-2.0)
                sc2m = pscr.tile([128, R], bf16, tag="sc2m")
                mkact(t, None, sc2m[:], sc[:], AF.Square)
                A = prow.tile([128, R], bf16, tag=f"A{t}")
                mkact(t, None, A[:], sc2m[:], AF.Copy, bias=SQ2, scale=-8.0 * SQ2)
                sf = pscr.tile([128, R], bf16, tag="sf")
                nc.vector.tensor_tensor(sf[:], sc[:], cs[:], OP.mult)
                sn = pscr.tile([128, R], bf16, tag="sn")
                nc.vector.tensor_tensor(sn[:], sf[:], st["i2n"][:], OP.mult)
                U = prow.tile([128, 5 * R], bf16, tag=f"U{t}")
                nc.vector.tensor_tensor(
                    sb(U, 0, [[R, 3], [1, R]]),
                    st["tv"][:].rearrange("p (c r) -> p c r", c=3),
                    sb(sn, 0, [[0, 3], [1, R]]), OP.mult)
                nc.vector.tensor_copy(U[:, 3 * R:5 * R], U[:, 0:2 * R])
                mA = pscr.tile([128, C], bf16, tag=f"mA{t}")
                nc.gpsimd.tensor_tensor(mA[:], A[:, 0:C], A[:, 1:1 + C], OP.mult)
                st["A"], st["U"], st["mA"] = A, U, mA

            def pair_a(t):
                st = state[t]
                A, U, tt = st["A"], st["U"], st["tt"]

                def A_at(row_off):
                    return sb(A, row_off, [[0, 3], [1, C]])

                def U_at(comp_rot, row_off):
                    return sb(U, comp_rot * R + row_off, [[R, 3], [1, C]])

                def TT_at(row_off):
                    return sb(tt, row_off, [[R, 3], [1, C]])

                cmC = lambda tl: sb(tl, 0, [[C, 3], [1, C]])

                # ----- translation part -----
                g1 = st["g1"]

                def G1(comp_rot):
                    return sb(g1, comp_rot * C, [[C, 3], [1, C]])

                p1 = pscr.tile([128, 3 * C], bf16, tag="p1")
                nc.vector.tensor_tensor(cmC(p1), U_at(1, 0), G1(2), OP.mult)
                p2 = pscr.tile([128, 3 * C], bf16, tag="p2")
                nc.vector.tensor_tensor(cmC(p2), U_at(2, 0), G1(1), OP.mult)
                b = pscr.tile([128, 5 * C], bf16, tag="b")
                nc.vector.tensor_tensor(cmC(b), p1[:], p2[:], OP.subtract)
                nc.vector.tensor_copy(b[:, 3 * C:5 * C], b[:, 0:2 * C])

                def B(comp_rot):
                    return sb(b, comp_rot * C, [[C, 3], [1, C]])

                q1 = pscr.tile([128, 3 * C], bf16, tag="p1")
                nc.vector.tensor_tensor(cmC(q1), U_at(1, 0), B(2), OP.mult)
                q2 = pscr.tile([128, 3 * C], bf16, tag="p2")
                nc.vector.tensor_tensor(cmC(q2), U_at(2, 0), B(1), OP.mult)
                cp = pscr.tile([128, 3 * C], bf16, tag="cp")
                nc.vector.tensor_tensor(cmC(cp), q1[:], q2[:], OP.subtract)
                m = pscr.tile([128, 3 * C], bf16, tag="m")
                nc.vector.tensor_tensor(cmC(m), A_at(0), B(0), OP.mult)

                g = pscr.tile([128, 3 * C], bf16, tag="gg")
                nc.vector.tensor_tensor(g[:], g1[:, 0:3 * C], st["gtt"][:],
                                        OP.subtract)
                gc = pscr.tile([128, 3 * C], bf16, tag="gc")
                nc.vector.tensor_tensor(gc[:], g[:], cp[:], OP.add)
                dfft = pscr.tile([128, 3 * C], bf16, tag="dfft")
                nc.vector.tensor_tensor(dfft[:], gc[:], m[:], OP.subtract)
                dump_t = pscr.tile([128, 3 * R], bf16, tag="sq")
                acc_t = pacc.tile([128, 1], f32, tag="acct")
                mkact(t, None, dump_t[:, 0:3 * C], dfft[:], AF.Abs,
                      accum_out=acc_t[:])
                acc_ts.append(acc_t)

                # ----- rotation part: qV = A0*U1 - A1*U0 - U0 x U1 (= 2*qv) --
                mA = st["mA"]
                mU = pscr.tile([128, 3 * C], bf16, tag="p1")
                nc.vector.tensor_tensor(cmC(mU), U_at(0, 0), U_at(0, 1), OP.mult)
                s1 = pscr.tile([128, C], bf16, tag="s1")
                nc.vector.tensor_tensor(s1[:], mU[:, 0:C], mU[:, C:2 * C], OP.add)
                s2 = pscr.tile([128, C], bf16, tag="s2")
                nc.vector.tensor_tensor(s2[:], s1[:], mU[:, 2 * C:3 * C], OP.add)
                qs2 = pscr.tile([128, C], bf16, tag="qs2")
                nc.vector.tensor_tensor(qs2[:], s2[:], mA[:], OP.add)

                pA = pscr.tile([128, 3 * C], bf16, tag="p1")
                nc.vector.tensor_tensor(cmC(pA), A_at(0), U_at(0, 1), OP.mult)
                pB = pscr.tile([128, 3 * C], bf16, tag="p2")
                nc.vector.tensor_tensor(cmC(pB), A_at(1), U_at(0, 0), OP.mult)
                w1 = pscr.tile([128, 3 * C], bf16, tag="w1")
                nc.vector.tensor_tensor(w1[:], pA[:], pB[:], OP.subtract)
                c1 = pscr.tile([128, 3 * C], bf16, tag="p1")
                nc.vector.tensor_tensor(cmC(c1), U_at(1, 0), U_at(2, 1), OP.mult)
                c2 = pscr.tile([128, 3 * C], bf16, tag="p2")
                nc.vector.tensor_tensor(cmC(c2), U_at(2, 0), U_at(1, 1), OP.mult)
                cr = pscr.tile([128, 3 * C], bf16, tag="cr")
                nc.vector.tensor_tensor(cr[:], c1[:], c2[:], OP.subtract)
                qV = pscr.tile([128, 3 * C], bf16, tag=f"qV{t}")
                nc.vector.tensor_tensor(qV[:], w1[:], cr[:], OP.subtract)

                qVsq = pscr.tile([128, 3 * C], bf16, tag="p1")
                mkact(t, None, qVsq[:], qV[:], AF.Square)
                nva = pscr.tile([128, C], bf16, tag="s1")
                nc.vector.tensor_tensor(nva[:], qVsq[:, 0:C], qVsq[:, C:2 * C],
                                        OP.add)
                nv2 = pscr.tile([128, C], bf16, tag="s2")
                nc.vector.tensor_tensor(nv2[:], nva[:], qVsq[:, 2 * C:3 * C],
                                        OP.add)

                lq = pscr.tile([128, C], f32, tag="lq")
                mkact(t, 'pairLE', lq[:], nv2[:], AF.Ln, bias=1e-16)
                rs = pscr.tile([128, C], bf16, tag=f"rs{t}")
                mkact(t, 'pairLE', rs[:], lq[:], AF.Exp, scale=-0.5)
                r2 = pscr.tile([128, C], bf16, tag=f"r2{t}")
                nc.vector.tensor_tensor(r2[:], qs2[:], rs[:], OP.mult)
                st["qV"], st["rs"], st["r2"] = qV, rs, r2

            def pair_b(t):
                st = state.pop(t)
                at = pscr.tile([128, C], f32, tag="at")
                mkact(t, 'pairTR', at[:], st["r2"][:], AF.Arctan, scale=-1.0)
                ratio = pscr.tile([128, C], bf16, tag="ratio")
                nc.vector.scalar_tensor_tensor(ratio[:], at[:], PI2, st["rs"][:],
                                               OP.add, OP.mult)
                ld = pscr.tile([128, 3 * C], bf16, tag="w1")
                nc.vector.tensor_tensor(
                    sb(ld, 0, [[C, 3], [1, C]]),
                    sb(st["qV"], 0, [[C, 3], [1, C]]),
                    sb(ratio, 0, [[0, 3], [1, C]]), OP.mult)
                ldiff = pscr.tile([128, 3 * C], bf16, tag="cr")
                nc.vector.tensor_tensor(ldiff[:], ld[:], st["gtq"][:], OP.subtract)
                dump_q = pscr.tile([128, 3 * R], bf16, tag="sq")
                acc_q = pacc.tile([128, 1], f32, tag="accq")
                mkact(t, None, dump_q[:, 0:3 * C], ldiff[:], AF.Abs,
                      accum_out=acc_q[:])
                acc_qs.append(acc_q)

            for t in range(NT):
                dma_phase(t)
            row_n2(0)
            row_lut(0)
            row_trig(0)
            for t in range(NT):
                early_g1(t)
            row_n2(1)
            row_n2(2)
            row_lut(1)
            row_lut(2)
            row_trig(1)
            row_trig(2)
            for t in range(NT):
                row_fin(t)
            for t in range(NT):
                pair_a(t)
            for t in range(NT):
                pair_b(t)

            # Chain LUT activations so same-table-set groups run contiguously
            # across tiles: 4 table loads total (rowLE, rowTR, pairLE, pairTR).
            order = [('rowLE', 0), ('rowTR', 0),
                     ('rowLE', 1), ('rowLE', 2),
                     ('rowTR', 1), ('rowTR', 2)]
            for g in ('pairLE', 'pairTR'):
                for ti in range(NT):
                    order.append((g, ti))
            seq = []
            for gname, ti in order:
                seq.extend(groups.get((ti, gname), []))
            for i in range(1, len(seq)):
                _add_dep_helper(seq[i].ins, seq[i - 1].ins, False,
                                "act table-set grouping")

            tot = pacc.tile([128, 2], f32, tag="tot")
            tmp_t = pacc.tile([128, 1], f32, tag="tmpt")
            nc.vector.tensor_tensor(tmp_t[:], acc_ts[0][:], acc_ts[1][:], OP.add)
            nc.vector.tensor_tensor(tot[:, 0:1], tmp_t[:], acc_ts[2][:], OP.add)
            tmp_q = pacc.tile([128, 1], f32, tag="tmpq")
            nc.vector.tensor_tensor(tmp_q[:], acc_qs[0][:], acc_qs[1][:], OP.add)
            nc.vector.tensor_tensor(tot[:, 1:2], tmp_q[:], acc_qs[2][:], OP.add)
            nc.sync.dma_start(out_h[:], tot[:])

    nc.compile()
    return nc


def _get_nc():
    if "nc" not in _BUILT:
        _BUILT["nc"] = _build()
    return _BUILT["nc"]


def run_device(pred, targ, trace=False):
    """pred: (1,T,6) f32, targ: (1,T-1,6) f32 -> (sum|dt|, sum|dq|, exec_ns)"""
    from concourse.bass_utils import run_bass_kernel_spmd

    nc = _get_nc()
    p = np.asarray(pred, dtype=np.float32).reshape(-1, 6)
    g = np.asarray(targ, dtype=np.float32).reshape(-1, 6)
    n_dup = ROWS_PAD - p.shape[0]
    p_pad = np.concatenate([p, np.repeat(p[-1:], n_dup, axis=0)], axis=0)
    g_pad = np.concatenate(
        [g, np.zeros((PAIRS_PAD - g.shape[0], 6), np.float32)], axis=0)

    in_maps = []
    for c in range(N_CORES):
        s = c * PPC
        in_maps.append({
            "pred": np.ascontiguousarray(p_pad[s:s + PPC + 1].T)
                     .astype(ml_dtypes.bfloat16).reshape(-1),
            "targ": np.ascontiguousarray(g_pad[s:s + PPC].T)
                     .astype(ml_dtypes.bfloat16).reshape(-1),
        })
    res = run_bass_kernel_spmd(nc, in_maps, core_ids=list(range(N_CORES)),
                               trace=trace)
    psum = np.stack([res.results[i]["out"] for i in range(N_CORES)])
    st = float(psum[:, :, 0].sum(dtype=np.float64))
    sq = float(psum[:, :, 1].sum(dtype=np.float64))
    return st, sq, res.exec_time_ns


def kernel(pred, targ, srx, srq):
    trace = bool(int(os.environ.get("VO_KERNEL_TRACE", "0")))
    st, sq, _ = run_device(pred, targ, trace=trace)
    t_loss = st / (3.0 * NPAIRS)
    q_loss = sq / (3.0 * NPAIRS)
    srx_v = float(np.asarray(srx).reshape(-1)[0])
    srq_v = float(np.asarray(srq).reshape(-1)[0])
    out = (np.exp(-srx_v) * t_loss + srx_v +
           np.exp(-srq_v) * q_loss + srq_v)
    return np.array([out], dtype=np.float32)


# revision 15
# speedup vs baseline: 1.1401x; 1.1401x over previous
"""Trainium2 Bass kernel for nn_AtLocPlusCriterion_VO.

loss = exp(-srx)*mean|vo_t - tg_t| + srx + exp(-srq)*mean|vo_q - tg_q| + srq
with vo = calc_vo_logq(pred[:-1], pred[1:]) (relative SE(3) pose, log-quaternion).

Sequence-parallel across 8 NeuronCores (1-row halo per shard). Inputs are
resharded host-side into component-major (SoA) bf16 planes so every on-device
vector op runs in the DVE 2x bf16 mode. Per core: 1956 pairs per SBUF
partition, 3 tiles of 652. Row phase (qexp via quarter-angle Sin LUT + Ln/Exp
roots) feeds a pair phase (rotation by two cross products, quaternion product,
log map via arctan) on VectorE; unary work (squares, LUTs, |x| + accumulate)
runs on ScalarE. Emission is phase-looped across tiles so scalar LUT work
groups into 4 activation-table loads (row Ln/Exp x3 tiles, row Sin x3,
pair Ln/Exp x3, pair Arctan x3). Mean-L1 reduces through per-partition
accumulators; host sums 8x[128,2].
"""
import os
import numpy as np
import ml_dtypes

N_CORES = 8
T_FULL = 2_000_000
NPAIRS = T_FULL - 1          # 1_999_999
D = 1956                     # pairs per partition per core
C = 652                      # pairs per tile (3 tiles)
NT = 3
R = C + 1
PPC = 128 * D                # 250_368 pairs per core
PAIRS_PAD = N_CORES * PPC    # 2_002_944
ROWS_PAD = PAIRS_PAD + 1

PRED_LEN = 6 * (PPC + 1)
TARG_LEN = 6 * PPC

LN4 = float(np.log(4.0))
LN4SQ2 = float(np.log(4.0 * np.sqrt(2.0)))   # i2n carries 4*sqrt2
PI2 = float(np.pi / 2.0)
SQ2 = float(np.sqrt(2.0))

_BUILT = {}


def _patch_act_tables():
    import concourse.bacc as bacc_mod
    import concourse.hw_specs as hw

    if getattr(bacc_mod, "_vo_tables_patched", False):
        return
    orig = hw.get_activation_tables

    def steered(arch, _orig=orig):
        from concourse import mybir as _mb
        AF = _mb.ActivationFunctionType
        t = {k: set(v) for k, v in _orig(arch).items()}
        # Keep all 24 entries (act_func_set_id indexes the original list);
        # drop ln/exp/arctan from the earlier sets so the table-load pass
        # resolves them to natural_log_exp_and_others / trig_and_small.
        t.get("natural_log", set()).discard(AF.Ln)
        t.get("exp_and_others", set()).discard(AF.Exp)
        t.get("sigmoid_and_others", set()).discard(AF.Arctan)
        return t

    bacc_mod.get_activation_tables = steered
    bacc_mod._vo_tables_patched = True


def _build():
    from concourse import bacc, tile, mybir
    from concourse.ap import AP
    from concourse.bass import _add_dep_helper

    _patch_act_tables()

    f32, bf16 = mybir.dt.float32, mybir.dt.bfloat16
    OP = mybir.AluOpType
    AF = mybir.ActivationFunctionType

    nc = bacc.Bacc("TRN2", target_bir_lowering=False, debug=False,
                   num_devices=N_CORES)
    pred_h = nc.declare_dram_parameter("pred", [PRED_LEN], bf16, isOutput=False)
    targ_h = nc.declare_dram_parameter("targ", [TARG_LEN], bf16, isOutput=False)
    out_h = nc.declare_dram_parameter("out", [128, 2], f32, isOutput=True)

    for v in (1e-16, -LN4, LN4SQ2, PI2):
        v = float(v)
        if (f32, v) not in nc.const_aps.aps:
            t = nc.alloc_sbuf_tensor(f"uconst-{v}", [128, 1], f32)
            nc.gpsimd.memset(t.ap(), v)
            nc.const_aps.aps[(f32, v)] = t.ap()
    nc.all_engine_barrier()

    PL_P = PPC + 1   # pred plane length
    PL_T = PPC       # targ plane length

    def sb(tile_, off, dims):
        base = tile_[:, :]
        return AP(base.tensor, base.offset + off,
                  [[base.ap.to_list()[0][0], 128]] + dims)

    acc_ts, acc_qs = [], []
    groups = {}  # (tile, name) -> list of act instructions

    with tile.TileContext(nc) as tc:

        def mkact(tile_i, group, *args, **kw):
            ins = nc.scalar.activation(*args, **kw)
            if group is not None:
                groups.setdefault((tile_i, group), []).append(ins)
            return ins

        with (
            tc.tile_pool(name="inp", bufs=1) as pin,
            tc.tile_pool(name="rowp", bufs=1) as prow,
            tc.tile_pool(name="scr", bufs=1) as pscr,
            tc.tile_pool(name="accp", bufs=8) as pacc,
        ):
            state = {}

            def dma_phase(t):
                # ---- DMA: component-major bf16 planes, all contiguous ----
                tv = pin.tile([128, 3 * R], bf16, tag=f"tv{t}")   # logq comps
                nc.sync.dma_start(
                    tv[:].rearrange("p (c r) -> p c r", c=3),
                    AP(pred_h, 3 * PL_P + t * C, [[D, 128], [PL_P, 3], [1, R]]))
                tt = pin.tile([128, 3 * R], bf16, tag=f"tt{t}")   # t comps
                nc.sync.dma_start(
                    tt[:].rearrange("p (c r) -> p c r", c=3),
                    AP(pred_h, t * C, [[D, 128], [PL_P, 3], [1, R]]))
                gtt = pin.tile([128, 3 * C], bf16, tag=f"gtt{t}")  # targ t
                nc.sync.dma_start(
                    gtt[:].rearrange("p (c r) -> p c r", c=3),
                    AP(targ_h, t * C, [[D, 128], [PL_T, 3], [1, C]]))
                gtq = pin.tile([128, 3 * C], bf16, tag=f"gtq{t}")  # targ q
                nc.sync.dma_start(
                    gtq[:].rearrange("p (c r) -> p c r", c=3),
                    AP(targ_h, 3 * PL_T + t * C, [[D, 128], [PL_T, 3], [1, C]]))
                state[t] = {"tv": tv, "tt": tt, "gtt": gtt, "gtq": gtq}

            def row_n2(t):
                st = state[t]
                sq = pscr.tile([128, 3 * R], bf16, tag="sq")
                mkact(t, None, sq[:], st["tv"][:], AF.Square)
                n2a = pscr.tile([128, R], bf16, tag="n2a")
                nc.vector.tensor_tensor(n2a[:], sq[:, 0:R], sq[:, R:2 * R], OP.add)
                n2 = pscr.tile([128, R], bf16, tag=f"n2{t}")
                nc.vector.tensor_tensor(n2[:], n2a[:], sq[:, 2 * R:3 * R], OP.add)
                st["n2"] = n2

            def row_lut(t):
                st = state[t]
                l = pscr.tile([128, R], f32, tag="l")
                mkact(t, 'rowLE', l[:], st["n2"][:], AF.Ln, bias=1e-16)
                n4 = pscr.tile([128, R], f32, tag=f"n4{t}")
                mkact(t, 'rowLE', n4[:], l[:], AF.Exp, bias=-LN4, scale=0.5)
                i2n = pscr.tile([128, R], bf16, tag=f"i2n{t}")
                mkact(t, 'rowLE', i2n[:], l[:], AF.Exp, bias=LN4SQ2, scale=-0.5)
                st["n4"], st["i2n"] = n4, i2n

            def row_trig(t):
                st = state[t]
                s4 = pscr.tile([128, R], bf16, tag=f"s4{t}")
                mkact(t, 'rowTR', s4[:], st["n4"][:], AF.Sin)
                c4 = pscr.tile([128, R], bf16, tag=f"c4{t}")
                mkact(t, 'rowTR', c4[:], st["n4"][:], AF.Sin, bias=PI2)
                st["s4"], st["c4"] = s4, c4

            def row_fin(t):
                st = state[t]
                sc = pscr.tile([128, R], bf16, tag="scx")
                nc.vector.tensor_tensor(sc[:], st["s4"][:], st["c4"][:], OP.mult)
                s4sq = pscr.tile([128, R], bf16, tag="s4sq")
                mkact(t, None, s4sq[:], st["s4"][:], AF.Square)
                cs = pscr.tile([128, R], bf16, tag="cs")
                mkact(t, None, cs[:], s4sq[:], AF.Copy, bias=1.0, scale=-2.0)
                sc2m = pscr.tile([128, R], bf16, tag="sc2m")
                mkact(t, None, sc2m[:], sc[:], AF.Square)
                A = prow.tile([128, R], bf16, tag=f"A{t}")
                mkact(t, None, A[:], sc2m[:], AF.Copy, bias=SQ2, scale=-8.0 * SQ2)
                sf = pscr.tile([128, R], bf16, tag="sf")
                nc.vector.tensor_tensor(sf[:], sc[:], cs[:], OP.mult)
                sn = pscr.tile([128, R], bf16, tag="sn")
                nc.vector.tensor_tensor(sn[:], sf[:], st["i2n"][:], OP.mult)
                U = prow.tile([128, 5 * R], bf16, tag=f"U{t}")
                nc.vector.tensor_tensor(
                    sb(U, 0, [[R, 3], [1, R]]),
                    st["tv"][:].rearrange("p (c r) -> p c r", c=3),
                    sb(sn, 0, [[0, 3], [1, R]]), OP.mult)
                nc.vector.tensor_copy(U[:, 3 * R:5 * R], U[:, 0:2 * R])
                st["A"], st["U"] = A, U

            def pair_a(t):
                st = state[t]
                A, U, tt = st["A"], st["U"], st["tt"]

                def A_at(row_off):
                    return sb(A, row_off, [[0, 3], [1, C]])

                def U_at(comp_rot, row_off):
                    return sb(U, comp_rot * R + row_off, [[R, 3], [1, C]])

                def TT_at(row_off):
                    return sb(tt, row_off, [[R, 3], [1, C]])

                cmC = lambda tl: sb(tl, 0, [[C, 3], [1, C]])

                # ----- translation part -----
                g1 = pscr.tile([128, 5 * C], bf16, tag="g1")
                nc.vector.tensor_tensor(cmC(g1), TT_at(1), TT_at(0), OP.subtract)
                nc.vector.tensor_copy(g1[:, 3 * C:5 * C], g1[:, 0:2 * C])

                def G1(comp_rot):
                    return sb(g1, comp_rot * C, [[C, 3], [1, C]])

                p1 = pscr.tile([128, 3 * C], bf16, tag="p1")
                nc.vector.tensor_tensor(cmC(p1), U_at(1, 0), G1(2), OP.mult)
                p2 = pscr.tile([128, 3 * C], bf16, tag="p2")
                nc.vector.tensor_tensor(cmC(p2), U_at(2, 0), G1(1), OP.mult)
                b = pscr.tile([128, 5 * C], bf16, tag="b")
                nc.vector.tensor_tensor(cmC(b), p1[:], p2[:], OP.subtract)
                nc.vector.tensor_copy(b[:, 3 * C:5 * C], b[:, 0:2 * C])

                def B(comp_rot):
                    return sb(b, comp_rot * C, [[C, 3], [1, C]])

                q1 = pscr.tile([128, 3 * C], bf16, tag="p1")
                nc.vector.tensor_tensor(cmC(q1), U_at(1, 0), B(2), OP.mult)
                q2 = pscr.tile([128, 3 * C], bf16, tag="p2")
                nc.vector.tensor_tensor(cmC(q2), U_at(2, 0), B(1), OP.mult)
                cp = pscr.tile([128, 3 * C], bf16, tag="cp")
                nc.vector.tensor_tensor(cmC(cp), q1[:], q2[:], OP.subtract)
                m = pscr.tile([128, 3 * C], bf16, tag="m")
                nc.vector.tensor_tensor(cmC(m), A_at(0), B(0), OP.mult)

                g = pscr.tile([128, 3 * C], bf16, tag="gg")
                nc.vector.tensor_tensor(g[:], g1[:, 0:3 * C], st["gtt"][:],
                                        OP.subtract)
                gc = pscr.tile([128, 3 * C], bf16, tag="gc")
                nc.vector.tensor_tensor(gc[:], g[:], cp[:], OP.add)
                dfft = pscr.tile([128, 3 * C], bf16, tag="dfft")
                nc.vector.tensor_tensor(dfft[:], gc[:], m[:], OP.subtract)
                dump_t = pscr.tile([128, 3 * R], bf16, tag="sq")
                acc_t = pacc.tile([128, 1], f32, tag="acct")
                mkact(t, None, dump_t[:, 0:3 * C], dfft[:], AF.Abs,
                      accum_out=acc_t[:])
                acc_ts.append(acc_t)

                # ----- rotation part: qV = A0*U1 - A1*U0 - U0 x U1 (= 2*qv) --
                mA = pscr.tile([128, C], bf16, tag="mA")
                nc.vector.tensor_tensor(mA[:], A[:, 0:C], A[:, 1:1 + C], OP.mult)
                mU = pscr.tile([128, 3 * C], bf16, tag="p1")
                nc.vector.tensor_tensor(cmC(mU), U_at(0, 0), U_at(0, 1), OP.mult)
                s1 = pscr.tile([128, C], bf16, tag="s1")
                nc.vector.tensor_tensor(s1[:], mU[:, 0:C], mU[:, C:2 * C], OP.add)
                s2 = pscr.tile([128, C], bf16, tag="s2")
                nc.vector.tensor_tensor(s2[:], s1[:], mU[:, 2 * C:3 * C], OP.add)
                qs2 = pscr.tile([128, C], bf16, tag="qs2")
                nc.vector.tensor_tensor(qs2[:], s2[:], mA[:], OP.add)

                pA = pscr.tile([128, 3 * C], bf16, tag="p1")
                nc.vector.tensor_tensor(cmC(pA), A_at(0), U_at(0, 1), OP.mult)
                pB = pscr.tile([128, 3 * C], bf16, tag="p2")
                nc.vector.tensor_tensor(cmC(pB), A_at(1), U_at(0, 0), OP.mult)
                w1 = pscr.tile([128, 3 * C], bf16, tag="w1")
                nc.vector.tensor_tensor(w1[:], pA[:], pB[:], OP.subtract)
                c1 = pscr.tile([128, 3 * C], bf16, tag="p1")
                nc.vector.tensor_tensor(cmC(c1), U_at(1, 0), U_at(2, 1), OP.mult)
                c2 = pscr.tile([128, 3 * C], bf16, tag="p2")
                nc.vector.tensor_tensor(cmC(c2), U_at(2, 0), U_at(1, 1), OP.mult)
                cr = pscr.tile([128, 3 * C], bf16, tag="cr")
                nc.vector.tensor_tensor(cr[:], c1[:], c2[:], OP.subtract)
                qV = pscr.tile([128, 3 * C], bf16, tag=f"qV{t}")
                nc.vector.tensor_tensor(qV[:], w1[:], cr[:], OP.subtract)

                qVsq = pscr.tile([128, 3 * C], bf16, tag="p1")
                mkact(t, None, qVsq[:], qV[:], AF.Square)
                nva = pscr.tile([128, C], bf16, tag="s1")
                nc.vector.tensor_tensor(nva[:], qVsq[:, 0:C], qVsq[:, C:2 * C],
                                        OP.add)
                nv2 = pscr.tile([128, C], bf16, tag="s2")
                nc.vector.tensor_tensor(nv2[:], nva[:], qVsq[:, 2 * C:3 * C],
                                        OP.add)

                lq = pscr.tile([128, C], f32, tag="lq")
                mkact(t, 'pairLE', lq[:], nv2[:], AF.Ln, bias=1e-16)
                rs = pscr.tile([128, C], bf16, tag=f"rs{t}")
                mkact(t, 'pairLE', rs[:], lq[:], AF.Exp, scale=-0.5)
                r2 = pscr.tile([128, C], bf16, tag=f"r2{t}")
                nc.vector.tensor_tensor(r2[:], qs2[:], rs[:], OP.mult)
                st["qV"], st["rs"], st["r2"] = qV, rs, r2

            def pair_b(t):
                st = state.pop(t)
                at = pscr.tile([128, C], f32, tag="at")
                mkact(t, 'pairTR', at[:], st["r2"][:], AF.Arctan, scale=-1.0)
                ratio = pscr.tile([128, C], bf16, tag="ratio")
                nc.vector.scalar_tensor_tensor(ratio[:], at[:], PI2, st["rs"][:],
                                               OP.add, OP.mult)
                ld = pscr.tile([128, 3 * C], bf16, tag="w1")
                nc.vector.tensor_tensor(
                    sb(ld, 0, [[C, 3], [1, C]]),
                    sb(st["qV"], 0, [[C, 3], [1, C]]),
                    sb(ratio, 0, [[0, 3], [1, C]]), OP.mult)
                ldiff = pscr.tile([128, 3 * C], bf16, tag="cr")
                nc.vector.tensor_tensor(ldiff[:], ld[:], st["gtq"][:], OP.subtract)
                dump_q = pscr.tile([128, 3 * R], bf16, tag="sq")
                acc_q = pacc.tile([128, 1], f32, tag="accq")
                mkact(t, None, dump_q[:, 0:3 * C], ldiff[:], AF.Abs,
                      accum_out=acc_q[:])
                acc_qs.append(acc_q)

            for t in range(NT):
                dma_phase(t)
            for t in range(NT):
                row_n2(t)
            for t in range(NT):
                row_lut(t)
            for t in range(NT):
                row_trig(t)
            for t in range(NT):
                row_fin(t)
            for t in range(NT):
                pair_a(t)
            for t in range(NT):
                pair_b(t)

            # Chain LUT activations so same-table-set groups run contiguously
            # across tiles: 4 table loads total (rowLE, rowTR, pairLE, pairTR).
            order = []
            for g in ('rowLE', 'rowTR', 'pairLE', 'pairTR'):
                for ti in range(NT):
                    order.append((g, ti))
            seq = []
            for gname, ti in order:
                seq.extend(groups.get((ti, gname), []))
            for i in range(1, len(seq)):
                _add_dep_helper(seq[i].ins, seq[i - 1].ins, False,
                                "act table-set grouping")

            tot = pacc.tile([128, 2], f32, tag="tot")
            tmp_t = pacc.tile([128, 1], f32, tag="tmpt")
            nc.vector.tensor_tensor(tmp_t[:], acc_ts[0][:], acc_ts[1][:], OP.add)
            nc.vector.tensor_tensor(tot[:, 0:1], tmp_t[:], acc_ts[2][:], OP.add)
            tmp_q = pacc.tile([128, 1], f32, tag="tmpq")
            nc.vector.tensor_tensor(tmp_q[:], acc_qs[0][:], acc_qs[1][:], OP.add)
            nc.vector.tensor_tensor(tot[:, 1:2], tmp_q[:], acc_qs[2][:], OP.add)
            nc.sync.dma_start(out_h[:], tot[:])

    nc.compile()
    return nc


def _get_nc():
    if "nc" not in _BUILT:
        _BUILT["nc"] = _build()
    return _BUILT["nc"]


def run_device(pred, targ, trace=False):
    """pred: (1,T,6) f32, targ: (1,T-1,6) f32 -> (sum|dt|, sum|dq|, exec_ns)"""
    from concourse.bass_utils import run_bass_kernel_spmd

    nc = _get_nc()
    p = np.asarray(pred, dtype=np.float32).reshape(-1, 6)
    g = np.asarray(targ, dtype=np.float32).reshape(-1, 6)
    n_dup = ROWS_PAD - p.shape[0]
    p_pad = np.concatenate([p, np.repeat(p[-1:], n_dup, axis=0)], axis=0)
    g_pad = np.concatenate(
        [g, np.zeros((PAIRS_PAD - g.shape[0], 6), np.float32)], axis=0)

    in_maps = []
    for c in range(N_CORES):
        s = c * PPC
        in_maps.append({
            "pred": np.ascontiguousarray(p_pad[s:s + PPC + 1].T)
                     .astype(ml_dtypes.bfloat16).reshape(-1),
            "targ": np.ascontiguousarray(g_pad[s:s + PPC].T)
                     .astype(ml_dtypes.bfloat16).reshape(-1),
        })
    res = run_bass_kernel_spmd(nc, in_maps, core_ids=list(range(N_CORES)),
                               trace=trace)
    psum = np.stack([res.results[i]["out"] for i in range(N_CORES)])
    st = float(psum[:, :, 0].sum(dtype=np.float64))
    sq = float(psum[:, :, 1].sum(dtype=np.float64))
    return st, sq, res.exec_time_ns


def kernel(pred, targ, srx, srq):
    trace = bool(int(os.environ.get("VO_KERNEL_TRACE", "0")))
    st, sq, _ = run_device(pred, targ, trace=trace)
    t_loss = st / (3.0 * NPAIRS)
    q_loss = sq / (3.0 * NPAIRS)
    srx_v = float(np.asarray(srx).reshape(-1)[0])
    srq_v = float(np.asarray(srq).reshape(-1)[0])
    out = (np.exp(-srx_v) * t_loss + srx_v +
           np.exp(-srq_v) * q_loss + srq_v)
    return np.array([out], dtype=np.float32)


# revision 16
# speedup vs baseline: 1.2040x; 1.0560x over previous
"""Trainium2 Bass kernel for nn_AtLocPlusCriterion_VO.

loss = exp(-srx)*mean|vo_t - tg_t| + srx + exp(-srq)*mean|vo_q - tg_q| + srq
with vo = calc_vo_logq(pred[:-1], pred[1:]) (relative SE(3) pose, log-quaternion).

Sequence-parallel across 8 NeuronCores (1-row halo per shard). Inputs are
resharded host-side into component-major (SoA) bf16 planes so every on-device
vector op runs in the DVE 2x bf16 mode. Per core: 1956 pairs per SBUF
partition, 3 tiles of 652. Row phase (qexp via quarter-angle Sin LUT + Ln/Exp
roots) feeds a pair phase (rotation by two cross products, quaternion product,
log map via arctan) on VectorE; unary work (squares, LUTs, |x| + accumulate)
runs on ScalarE. Emission is phase-looped across tiles so scalar LUT work
groups into 4 activation-table loads (row Ln/Exp x3 tiles, row Sin x3,
pair Ln/Exp x3, pair Arctan x3). Mean-L1 reduces through per-partition
accumulators; host sums 8x[128,2].
"""
import os
import numpy as np
import ml_dtypes

N_CORES = 8
T_FULL = 2_000_000
NPAIRS = T_FULL - 1          # 1_999_999
D = 1956                     # pairs per partition per core
C = 652                      # pairs per tile (3 tiles)
NT = 3
R = C + 1
PPC = 128 * D                # 250_368 pairs per core
PAIRS_PAD = N_CORES * PPC    # 2_002_944
ROWS_PAD = PAIRS_PAD + 1

PRED_LEN = 6 * (PPC + 1)
TARG_LEN = 6 * PPC

LN4 = float(np.log(4.0))
LN4SQ2 = float(np.log(4.0 * np.sqrt(2.0)))   # i2n carries 4*sqrt2
PI2 = float(np.pi / 2.0)
SQ2 = float(np.sqrt(2.0))

_BUILT = {}


def _patch_act_tables():
    import concourse.bacc as bacc_mod
    import concourse.hw_specs as hw

    if getattr(bacc_mod, "_vo_tables_patched", False):
        return
    orig = hw.get_activation_tables

    def steered(arch, _orig=orig):
        from concourse import mybir as _mb
        AF = _mb.ActivationFunctionType
        t = {k: set(v) for k, v in _orig(arch).items()}
        # Keep all 24 entries (act_func_set_id indexes the original list);
        # drop ln/exp/arctan from the earlier sets so the table-load pass
        # resolves them to natural_log_exp_and_others / trig_and_small.
        t.get("natural_log", set()).discard(AF.Ln)
        t.get("exp_and_others", set()).discard(AF.Exp)
        t.get("sigmoid_and_others", set()).discard(AF.Arctan)
        return t

    bacc_mod.get_activation_tables = steered
    bacc_mod._vo_tables_patched = True


def _build():
    from concourse import bacc, tile, mybir
    from concourse.ap import AP
    from concourse.bass import _add_dep_helper

    _patch_act_tables()

    f32, bf16 = mybir.dt.float32, mybir.dt.bfloat16
    OP = mybir.AluOpType
    AF = mybir.ActivationFunctionType

    nc = bacc.Bacc("TRN2", target_bir_lowering=False, debug=False,
                   num_devices=N_CORES)
    pred_h = nc.declare_dram_parameter("pred", [PRED_LEN], bf16, isOutput=False)
    targ_h = nc.declare_dram_parameter("targ", [TARG_LEN], bf16, isOutput=False)
    out_h = nc.declare_dram_parameter("out", [128, 2], f32, isOutput=True)

    for v in (1e-16, -LN4, LN4SQ2, PI2):
        v = float(v)
        if (f32, v) not in nc.const_aps.aps:
            t = nc.alloc_sbuf_tensor(f"uconst-{v}", [128, 1], f32)
            nc.gpsimd.memset(t.ap(), v)
            nc.const_aps.aps[(f32, v)] = t.ap()
    nc.all_engine_barrier()

    PL_P = PPC + 1   # pred plane length
    PL_T = PPC       # targ plane length

    def sb(tile_, off, dims):
        base = tile_[:, :]
        return AP(base.tensor, base.offset + off,
                  [[base.ap.to_list()[0][0], 128]] + dims)

    acc_ts, acc_qs = [], []
    groups = {}  # (tile, name) -> list of act instructions

    with tile.TileContext(nc) as tc:

        def mkact(tile_i, group, *args, **kw):
            ins = nc.scalar.activation(*args, **kw)
            if group is not None:
                groups.setdefault((tile_i, group), []).append(ins)
            return ins

        with (
            tc.tile_pool(name="inp", bufs=1) as pin,
            tc.tile_pool(name="rowp", bufs=1) as prow,
            tc.tile_pool(name="scr", bufs=1) as pscr,
            tc.tile_pool(name="accp", bufs=8) as pacc,
        ):
            state = {}

            def dma_phase(t):
                # ---- DMA: component-major bf16 planes, all contiguous ----
                tv = pin.tile([128, 3 * R], bf16, tag=f"tv{t}")   # logq comps
                nc.sync.dma_start(
                    tv[:].rearrange("p (c r) -> p c r", c=3),
                    AP(pred_h, 3 * PL_P + t * C, [[D, 128], [PL_P, 3], [1, R]]))
                tt = pin.tile([128, 3 * R], bf16, tag=f"tt{t}")   # t comps
                nc.sync.dma_start(
                    tt[:].rearrange("p (c r) -> p c r", c=3),
                    AP(pred_h, t * C, [[D, 128], [PL_P, 3], [1, R]]))
                gtt = pin.tile([128, 3 * C], bf16, tag=f"gtt{t}")  # targ t
                nc.sync.dma_start(
                    gtt[:].rearrange("p (c r) -> p c r", c=3),
                    AP(targ_h, t * C, [[D, 128], [PL_T, 3], [1, C]]))
                gtq = pin.tile([128, 3 * C], bf16, tag=f"gtq{t}")  # targ q
                nc.sync.dma_start(
                    gtq[:].rearrange("p (c r) -> p c r", c=3),
                    AP(targ_h, 3 * PL_T + t * C, [[D, 128], [PL_T, 3], [1, C]]))
                state[t] = {"tv": tv, "tt": tt, "gtt": gtt, "gtq": gtq}

            def row_n2(t):
                st = state[t]
                sq = pscr.tile([128, 3 * R], bf16, tag="sq")
                mkact(t, None, sq[:], st["tv"][:], AF.Square)
                n2a = pscr.tile([128, R], bf16, tag="n2a")
                nc.vector.tensor_tensor(n2a[:], sq[:, 0:R], sq[:, R:2 * R], OP.add)
                n2 = pscr.tile([128, R], bf16, tag=f"n2{t}")
                nc.vector.tensor_tensor(n2[:], n2a[:], sq[:, 2 * R:3 * R], OP.add)
                st["n2"] = n2

            def early_g1(t):
                # translation deltas: independent of the row LUT chain, runs
                # on VectorE while ScalarE does table loads + Ln/Exp/Sin.
                st = state[t]
                tt = st["tt"]
                g1 = pscr.tile([128, 5 * C], bf16, tag=f"g1{t}")
                nc.vector.tensor_tensor(
                    sb(g1, 0, [[C, 3], [1, C]]),
                    sb(tt, 1, [[R, 3], [1, C]]),
                    sb(tt, 0, [[R, 3], [1, C]]), OP.subtract)
                nc.vector.tensor_copy(g1[:, 3 * C:5 * C], g1[:, 0:2 * C])
                st["g1"] = g1

            def row_lut(t):
                st = state[t]
                l = pscr.tile([128, R], f32, tag="l")
                mkact(t, 'rowLE', l[:], st["n2"][:], AF.Ln, bias=1e-16)
                n4 = pscr.tile([128, R], f32, tag=f"n4{t}")
                mkact(t, 'rowLE', n4[:], l[:], AF.Exp, bias=-LN4, scale=0.5)
                i2n = pscr.tile([128, R], bf16, tag=f"i2n{t}")
                mkact(t, 'rowLE', i2n[:], l[:], AF.Exp, bias=LN4SQ2, scale=-0.5)
                st["n4"], st["i2n"] = n4, i2n

            def row_trig(t):
                st = state[t]
                s4 = pscr.tile([128, R], bf16, tag=f"s4{t}")
                mkact(t, 'rowTR', s4[:], st["n4"][:], AF.Sin)
                c4 = pscr.tile([128, R], bf16, tag=f"c4{t}")
                mkact(t, 'rowTR', c4[:], st["n4"][:], AF.Sin, bias=PI2)
                st["s4"], st["c4"] = s4, c4

            def row_fin(t):
                st = state[t]
                sc = pscr.tile([128, R], bf16, tag="scx")
                nc.vector.tensor_tensor(sc[:], st["s4"][:], st["c4"][:], OP.mult)
                s4sq = pscr.tile([128, R], bf16, tag="s4sq")
                mkact(t, None, s4sq[:], st["s4"][:], AF.Square)
                cs = pscr.tile([128, R], bf16, tag="cs")
                mkact(t, None, cs[:], s4sq[:], AF.Copy, bias=1.0, scale=-2.0)
                sc2m = pscr.tile([128, R], bf16, tag="sc2m")
                mkact(t, None, sc2m[:], sc[:], AF.Square)
                A = prow.tile([128, R], bf16, tag=f"A{t}")
                mkact(t, None, A[:], sc2m[:], AF.Copy, bias=SQ2, scale=-8.0 * SQ2)
                sf = pscr.tile([128, R], bf16, tag="sf")
                nc.vector.tensor_tensor(sf[:], sc[:], cs[:], OP.mult)
                sn = pscr.tile([128, R], bf16, tag="sn")
                nc.vector.tensor_tensor(sn[:], sf[:], st["i2n"][:], OP.mult)
                U = prow.tile([128, 5 * R], bf16, tag=f"U{t}")
                nc.vector.tensor_tensor(
                    sb(U, 0, [[R, 3], [1, R]]),
                    st["tv"][:].rearrange("p (c r) -> p c r", c=3),
                    sb(sn, 0, [[0, 3], [1, R]]), OP.mult)
                nc.vector.tensor_copy(U[:, 3 * R:5 * R], U[:, 0:2 * R])
                st["A"], st["U"] = A, U

            def pair_a(t):
                st = state[t]
                A, U, tt = st["A"], st["U"], st["tt"]

                def A_at(row_off):
                    return sb(A, row_off, [[0, 3], [1, C]])

                def U_at(comp_rot, row_off):
                    return sb(U, comp_rot * R + row_off, [[R, 3], [1, C]])

                def TT_at(row_off):
                    return sb(tt, row_off, [[R, 3], [1, C]])

                cmC = lambda tl: sb(tl, 0, [[C, 3], [1, C]])

                # ----- translation part -----
                if "g1" in st:
                    g1 = st["g1"]
                else:
                    g1 = pscr.tile([128, 5 * C], bf16, tag="g10")
                    nc.vector.tensor_tensor(cmC(g1), TT_at(1), TT_at(0),
                                            OP.subtract)
                    nc.vector.tensor_copy(g1[:, 3 * C:5 * C], g1[:, 0:2 * C])

                def G1(comp_rot):
                    return sb(g1, comp_rot * C, [[C, 3], [1, C]])

                p1 = pscr.tile([128, 3 * C], bf16, tag="p1")
                nc.vector.tensor_tensor(cmC(p1), U_at(1, 0), G1(2), OP.mult)
                p2 = pscr.tile([128, 3 * C], bf16, tag="p2")
                nc.vector.tensor_tensor(cmC(p2), U_at(2, 0), G1(1), OP.mult)
                b = pscr.tile([128, 5 * C], bf16, tag="b")
                nc.vector.tensor_tensor(cmC(b), p1[:], p2[:], OP.subtract)
                nc.vector.tensor_copy(b[:, 3 * C:5 * C], b[:, 0:2 * C])

                def B(comp_rot):
                    return sb(b, comp_rot * C, [[C, 3], [1, C]])

                q1 = pscr.tile([128, 3 * C], bf16, tag="p1")
                nc.vector.tensor_tensor(cmC(q1), U_at(1, 0), B(2), OP.mult)
                q2 = pscr.tile([128, 3 * C], bf16, tag="p2")
                nc.vector.tensor_tensor(cmC(q2), U_at(2, 0), B(1), OP.mult)
                cp = pscr.tile([128, 3 * C], bf16, tag="cp")
                nc.vector.tensor_tensor(cmC(cp), q1[:], q2[:], OP.subtract)
                m = pscr.tile([128, 3 * C], bf16, tag="m")
                nc.vector.tensor_tensor(cmC(m), A_at(0), B(0), OP.mult)

                g = pscr.tile([128, 3 * C], bf16, tag="gg")
                nc.vector.tensor_tensor(g[:], g1[:, 0:3 * C], st["gtt"][:],
                                        OP.subtract)
                gc = pscr.tile([128, 3 * C], bf16, tag="gc")
                nc.vector.tensor_tensor(gc[:], g[:], cp[:], OP.add)
                dfft = pscr.tile([128, 3 * C], bf16, tag="dfft")
                nc.vector.tensor_tensor(dfft[:], gc[:], m[:], OP.subtract)
                dump_t = pscr.tile([128, 3 * R], bf16, tag="sq")
                acc_t = pacc.tile([128, 1], f32, tag="acct")
                mkact(t, None, dump_t[:, 0:3 * C], dfft[:], AF.Abs,
                      accum_out=acc_t[:])
                acc_ts.append(acc_t)

                # ----- rotation part: qV = A0*U1 - A1*U0 - U0 x U1 (= 2*qv) --
                mA = pscr.tile([128, C], bf16, tag="mA")
                nc.vector.tensor_tensor(mA[:], A[:, 0:C], A[:, 1:1 + C], OP.mult)
                mU = pscr.tile([128, 3 * C], bf16, tag="p1")
                nc.vector.tensor_tensor(cmC(mU), U_at(0, 0), U_at(0, 1), OP.mult)
                s1 = pscr.tile([128, C], bf16, tag="s1")
                nc.vector.tensor_tensor(s1[:], mU[:, 0:C], mU[:, C:2 * C], OP.add)
                s2 = pscr.tile([128, C], bf16, tag="s2")
                nc.vector.tensor_tensor(s2[:], s1[:], mU[:, 2 * C:3 * C], OP.add)
                qs2 = pscr.tile([128, C], bf16, tag="qs2")
                nc.vector.tensor_tensor(qs2[:], s2[:], mA[:], OP.add)

                pA = pscr.tile([128, 3 * C], bf16, tag="p1")
                nc.vector.tensor_tensor(cmC(pA), A_at(0), U_at(0, 1), OP.mult)
                pB = pscr.tile([128, 3 * C], bf16, tag="p2")
                nc.vector.tensor_tensor(cmC(pB), A_at(1), U_at(0, 0), OP.mult)
                w1 = pscr.tile([128, 3 * C], bf16, tag="w1")
                nc.vector.tensor_tensor(w1[:], pA[:], pB[:], OP.subtract)
                c1 = pscr.tile([128, 3 * C], bf16, tag="p1")
                nc.vector.tensor_tensor(cmC(c1), U_at(1, 0), U_at(2, 1), OP.mult)
                c2 = pscr.tile([128, 3 * C], bf16, tag="p2")
                nc.vector.tensor_tensor(cmC(c2), U_at(2, 0), U_at(1, 1), OP.mult)
                cr = pscr.tile([128, 3 * C], bf16, tag="cr")
                nc.vector.tensor_tensor(cr[:], c1[:], c2[:], OP.subtract)
                qV = pscr.tile([128, 3 * C], bf16, tag=f"qV{t}")
                nc.vector.tensor_tensor(qV[:], w1[:], cr[:], OP.subtract)

                qVsq = pscr.tile([128, 3 * C], bf16, tag="p1")
                mkact(t, None, qVsq[:], qV[:], AF.Square)
                nva = pscr.tile([128, C], bf16, tag="s1")
                nc.vector.tensor_tensor(nva[:], qVsq[:, 0:C], qVsq[:, C:2 * C],
                                        OP.add)
                nv2 = pscr.tile([128, C], bf16, tag="s2")
                nc.vector.tensor_tensor(nv2[:], nva[:], qVsq[:, 2 * C:3 * C],
                                        OP.add)

                lq = pscr.tile([128, C], f32, tag="lq")
                mkact(t, 'pairLE', lq[:], nv2[:], AF.Ln, bias=1e-16)
                rs = pscr.tile([128, C], bf16, tag=f"rs{t}")
                mkact(t, 'pairLE', rs[:], lq[:], AF.Exp, scale=-0.5)
                r2 = pscr.tile([128, C], bf16, tag=f"r2{t}")
                nc.vector.tensor_tensor(r2[:], qs2[:], rs[:], OP.mult)
                st["qV"], st["rs"], st["r2"] = qV, rs, r2

            def pair_b(t):
                st = state.pop(t)
                at = pscr.tile([128, C], f32, tag="at")
                mkact(t, 'pairTR', at[:], st["r2"][:], AF.Arctan, scale=-1.0)
                ratio = pscr.tile([128, C], bf16, tag="ratio")
                nc.vector.scalar_tensor_tensor(ratio[:], at[:], PI2, st["rs"][:],
                                               OP.add, OP.mult)
                ld = pscr.tile([128, 3 * C], bf16, tag="w1")
                nc.vector.tensor_tensor(
                    sb(ld, 0, [[C, 3], [1, C]]),
                    sb(st["qV"], 0, [[C, 3], [1, C]]),
                    sb(ratio, 0, [[0, 3], [1, C]]), OP.mult)
                ldiff = pscr.tile([128, 3 * C], bf16, tag="cr")
                nc.vector.tensor_tensor(ldiff[:], ld[:], st["gtq"][:], OP.subtract)
                dump_q = pscr.tile([128, 3 * R], bf16, tag="sq")
                acc_q = pacc.tile([128, 1], f32, tag="accq")
                mkact(t, None, dump_q[:, 0:3 * C], ldiff[:], AF.Abs,
                      accum_out=acc_q[:])
                acc_qs.append(acc_q)

            for t in range(NT):
                dma_phase(t)
            row_n2(0)
            row_lut(0)
            row_trig(0)
            row_n2(1)
            row_n2(2)
            early_g1(0)
            early_g1(1)
            row_lut(1)
            row_lut(2)
            row_trig(1)
            row_trig(2)
            for t in range(NT):
                row_fin(t)
            for t in range(NT):
                pair_a(t)
            for t in range(NT):
                pair_b(t)

            # Chain LUT activations so same-table-set groups run contiguously
            # across tiles: 4 table loads total (rowLE, rowTR, pairLE, pairTR).
            order = [('rowLE', 0), ('rowTR', 0),
                     ('rowLE', 1), ('rowLE', 2),
                     ('rowTR', 1), ('rowTR', 2)]
            for g in ('pairLE', 'pairTR'):
                for ti in range(NT):
                    order.append((g, ti))
            seq = []
            for gname, ti in order:
                seq.extend(groups.get((ti, gname), []))
            for i in range(1, len(seq)):
                _add_dep_helper(seq[i].ins, seq[i - 1].ins, False,
                                "act table-set grouping")

            tot = pacc.tile([128, 2], f32, tag="tot")
            tmp_t = pacc.tile([128, 1], f32, tag="tmpt")
            nc.vector.tensor_tensor(tmp_t[:], acc_ts[0][:], acc_ts[1][:], OP.add)
            nc.vector.tensor_tensor(tot[:, 0:1], tmp_t[:], acc_ts[2][:], OP.add)
            tmp_q = pacc.tile([128, 1], f32, tag="tmpq")
            nc.vector.tensor_tensor(tmp_q[:], acc_qs[0][:], acc_qs[1][:], OP.add)
            nc.vector.tensor_tensor(tot[:, 1:2], tmp_q[:], acc_qs[2][:], OP.add)
            nc.sync.dma_start(out_h[:], tot[:])

    nc.compile()
    return nc


def _get_nc():
    if "nc" not in _BUILT:
        _BUILT["nc"] = _build()
    return _BUILT["nc"]


def run_device(pred, targ, trace=False):
    """pred: (1,T,6) f32, targ: (1,T-1,6) f32 -> (sum|dt|, sum|dq|, exec_ns)"""
    from concourse.bass_utils import run_bass_kernel_spmd

    nc = _get_nc()
    p = np.asarray(pred, dtype=np.float32).reshape(-1, 6)
    g = np.asarray(targ, dtype=np.float32).reshape(-1, 6)
    n_dup = ROWS_PAD - p.shape[0]
    p_pad = np.concatenate([p, np.repeat(p[-1:], n_dup, axis=0)], axis=0)
    g_pad = np.concatenate(
        [g, np.zeros((PAIRS_PAD - g.shape[0], 6), np.float32)], axis=0)

    in_maps = []
    for c in range(N_CORES):
        s = c * PPC
        in_maps.append({
            "pred": np.ascontiguousarray(p_pad[s:s + PPC + 1].T)
                     .astype(ml_dtypes.bfloat16).reshape(-1),
            "targ": np.ascontiguousarray(g_pad[s:s + PPC].T)
                     .astype(ml_dtypes.bfloat16).reshape(-1),
        })
    res = run_bass_kernel_spmd(nc, in_maps, core_ids=list(range(N_CORES)),
                               trace=trace)
    psum = np.stack([res.results[i]["out"] for i in range(N_CORES)])
    st = float(psum[:, :, 0].sum(dtype=np.float64))
    sq = float(psum[:, :, 1].sum(dtype=np.float64))
    return st, sq, res.exec_time_ns


def kernel(pred, targ, srx, srq):
    trace = bool(int(os.environ.get("VO_KERNEL_TRACE", "0")))
    st, sq, _ = run_device(pred, targ, trace=trace)
    t_loss = st / (3.0 * NPAIRS)
    q_loss = sq / (3.0 * NPAIRS)
    srx_v = float(np.asarray(srx).reshape(-1)[0])
    srq_v = float(np.asarray(srq).reshape(-1)[0])
    out = (np.exp(-srx_v) * t_loss + srx_v +
           np.exp(-srq_v) * q_loss + srq_v)
    return np.array([out], dtype=np.float32)


# revision 17
# speedup vs baseline: 1.2131x; 1.0076x over previous
"""Trainium2 Bass kernel for nn_AtLocPlusCriterion_VO.

loss = exp(-srx)*mean|vo_t - tg_t| + srx + exp(-srq)*mean|vo_q - tg_q| + srq
with vo = calc_vo_logq(pred[:-1], pred[1:]) (relative SE(3) pose, log-quaternion).

Sequence-parallel across 8 NeuronCores (1-row halo per shard). Inputs are
resharded host-side into component-major (SoA) bf16 planes so every on-device
vector op runs in the DVE 2x bf16 mode. Per core: 1956 pairs per SBUF
partition, 3 tiles of 652. Row phase (qexp via quarter-angle Sin LUT + Ln/Exp
roots) feeds a pair phase (rotation by two cross products, quaternion product,
log map via arctan) on VectorE; unary work (squares, LUTs, |x| + accumulate)
runs on ScalarE. Emission is phase-looped across tiles so scalar LUT work
groups into 4 activation-table loads (row Ln/Exp x3 tiles, row Sin x3,
pair Ln/Exp x3, pair Arctan x3). Mean-L1 reduces through per-partition
accumulators; host sums 8x[128,2].
"""
import os
import numpy as np
import ml_dtypes

N_CORES = 8
T_FULL = 2_000_000
NPAIRS = T_FULL - 1          # 1_999_999
D = 1956                     # pairs per partition per core
C = 652                      # pairs per tile (3 tiles)
NT = 3
R = C + 1
PPC = 128 * D                # 250_368 pairs per core
PAIRS_PAD = N_CORES * PPC    # 2_002_944
ROWS_PAD = PAIRS_PAD + 1

PRED_LEN = 6 * (PPC + 1)
TARG_LEN = 6 * PPC

LN4 = float(np.log(4.0))
LN4SQ2 = float(np.log(4.0 * np.sqrt(2.0)))   # i2n carries 4*sqrt2
PI2 = float(np.pi / 2.0)
SQ2 = float(np.sqrt(2.0))

_BUILT = {}


def _patch_act_tables():
    import concourse.bacc as bacc_mod
    import concourse.hw_specs as hw

    if getattr(bacc_mod, "_vo_tables_patched", False):
        return
    orig = hw.get_activation_tables

    def steered(arch, _orig=orig):
        from concourse import mybir as _mb
        AF = _mb.ActivationFunctionType
        t = {k: set(v) for k, v in _orig(arch).items()}
        # Keep all 24 entries (act_func_set_id indexes the original list);
        # drop ln/exp/arctan from the earlier sets so the table-load pass
        # resolves them to natural_log_exp_and_others / trig_and_small.
        t.get("natural_log", set()).discard(AF.Ln)
        t.get("exp_and_others", set()).discard(AF.Exp)
        t.get("sigmoid_and_others", set()).discard(AF.Arctan)
        return t

    bacc_mod.get_activation_tables = steered
    bacc_mod._vo_tables_patched = True


def _build():
    from concourse import bacc, tile, mybir
    from concourse.ap import AP
    from concourse.bass import _add_dep_helper

    _patch_act_tables()

    f32, bf16 = mybir.dt.float32, mybir.dt.bfloat16
    OP = mybir.AluOpType
    AF = mybir.ActivationFunctionType

    nc = bacc.Bacc("TRN2", target_bir_lowering=False, debug=False,
                   num_devices=N_CORES)
    pred_h = nc.declare_dram_parameter("pred", [PRED_LEN], bf16, isOutput=False)
    targ_h = nc.declare_dram_parameter("targ", [TARG_LEN], bf16, isOutput=False)
    out_h = nc.declare_dram_parameter("out", [128, 2], f32, isOutput=True)

    for v in (1e-16, -LN4, LN4SQ2, PI2):
        v = float(v)
        if (f32, v) not in nc.const_aps.aps:
            t = nc.alloc_sbuf_tensor(f"uconst-{v}", [128, 1], f32)
            nc.gpsimd.memset(t.ap(), v)
            nc.const_aps.aps[(f32, v)] = t.ap()
    nc.all_engine_barrier()

    PL_P = PPC + 1   # pred plane length
    PL_T = PPC       # targ plane length

    def sb(tile_, off, dims):
        base = tile_[:, :]
        return AP(base.tensor, base.offset + off,
                  [[base.ap.to_list()[0][0], 128]] + dims)

    acc_ts, acc_qs = [], []
    groups = {}  # (tile, name) -> list of act instructions

    with tile.TileContext(nc) as tc:

        def mkact(tile_i, group, *args, **kw):
            ins = nc.scalar.activation(*args, **kw)
            if group is not None:
                groups.setdefault((tile_i, group), []).append(ins)
            return ins

        with (
            tc.tile_pool(name="inp", bufs=1) as pin,
            tc.tile_pool(name="rowp", bufs=1) as prow,
            tc.tile_pool(name="scr", bufs=1) as pscr,
            tc.tile_pool(name="accp", bufs=8) as pacc,
        ):
            state = {}

            def dma_phase(t):
                # ---- DMA: component-major bf16 planes, all contiguous ----
                tv = pin.tile([128, 3 * R], bf16, tag=f"tv{t}")   # logq comps
                nc.sync.dma_start(
                    tv[:].rearrange("p (c r) -> p c r", c=3),
                    AP(pred_h, 3 * PL_P + t * C, [[D, 128], [PL_P, 3], [1, R]]))
                tt = pin.tile([128, 3 * R], bf16, tag=f"tt{t}")   # t comps
                nc.sync.dma_start(
                    tt[:].rearrange("p (c r) -> p c r", c=3),
                    AP(pred_h, t * C, [[D, 128], [PL_P, 3], [1, R]]))
                gtt = pin.tile([128, 3 * C], bf16, tag=f"gtt{t}")  # targ t
                nc.sync.dma_start(
                    gtt[:].rearrange("p (c r) -> p c r", c=3),
                    AP(targ_h, t * C, [[D, 128], [PL_T, 3], [1, C]]))
                gtq = pin.tile([128, 3 * C], bf16, tag=f"gtq{t}")  # targ q
                nc.sync.dma_start(
                    gtq[:].rearrange("p (c r) -> p c r", c=3),
                    AP(targ_h, 3 * PL_T + t * C, [[D, 128], [PL_T, 3], [1, C]]))
                state[t] = {"tv": tv, "tt": tt, "gtt": gtt, "gtq": gtq}

            def row_n2(t):
                st = state[t]
                sq = pscr.tile([128, 3 * R], bf16, tag="sq")
                if t == 0:
                    mkact(t, None, sq[:], st["tv"][:], AF.Square)
                else:
                    nc.vector.tensor_tensor(sq[:], st["tv"][:], st["tv"][:],
                                            OP.mult)
                n2a = pscr.tile([128, R], bf16, tag="n2a")
                nc.vector.tensor_tensor(n2a[:], sq[:, 0:R], sq[:, R:2 * R], OP.add)
                n2 = pscr.tile([128, R], bf16, tag=f"n2{t}")
                nc.vector.tensor_tensor(n2[:], n2a[:], sq[:, 2 * R:3 * R], OP.add)
                st["n2"] = n2

            def early_g1(t):
                # translation deltas: independent of the row LUT chain, runs
                # on VectorE while ScalarE does table loads + Ln/Exp/Sin.
                st = state[t]
                tt = st["tt"]
                g1 = pscr.tile([128, 5 * C], bf16, tag=f"g1{t}")
                nc.vector.tensor_tensor(
                    sb(g1, 0, [[C, 3], [1, C]]),
                    sb(tt, 1, [[R, 3], [1, C]]),
                    sb(tt, 0, [[R, 3], [1, C]]), OP.subtract)
                nc.vector.tensor_copy(g1[:, 3 * C:5 * C], g1[:, 0:2 * C])
                st["g1"] = g1

            def row_lut(t):
                st = state[t]
                l = pscr.tile([128, R], f32, tag="l")
                mkact(t, 'rowLE', l[:], st["n2"][:], AF.Ln, bias=1e-16)
                n4 = pscr.tile([128, R], f32, tag=f"n4{t}")
                mkact(t, 'rowLE', n4[:], l[:], AF.Exp, bias=-LN4, scale=0.5)
                i2n = pscr.tile([128, R], bf16, tag=f"i2n{t}")
                mkact(t, 'rowLE', i2n[:], l[:], AF.Exp, bias=LN4SQ2, scale=-0.5)
                st["n4"], st["i2n"] = n4, i2n

            def row_trig(t):
                st = state[t]
                s4 = pscr.tile([128, R], bf16, tag=f"s4{t}")
                mkact(t, 'rowTR', s4[:], st["n4"][:], AF.Sin)
                c4 = pscr.tile([128, R], bf16, tag=f"c4{t}")
                mkact(t, 'rowTR', c4[:], st["n4"][:], AF.Sin, bias=PI2)
                st["s4"], st["c4"] = s4, c4

            def row_fin(t):
                st = state[t]
                sc = pscr.tile([128, R], bf16, tag="scx")
                nc.vector.tensor_tensor(sc[:], st["s4"][:], st["c4"][:], OP.mult)
                s4sq = pscr.tile([128, R], bf16, tag="s4sq")
                mkact(t, None, s4sq[:], st["s4"][:], AF.Square)
                cs = pscr.tile([128, R], bf16, tag="cs")
                mkact(t, None, cs[:], s4sq[:], AF.Copy, bias=1.0, scale=-2.0)
                sc2m = pscr.tile([128, R], bf16, tag="sc2m")
                mkact(t, None, sc2m[:], sc[:], AF.Square)
                A = prow.tile([128, R], bf16, tag=f"A{t}")
                mkact(t, None, A[:], sc2m[:], AF.Copy, bias=SQ2, scale=-8.0 * SQ2)
                sf = pscr.tile([128, R], bf16, tag="sf")
                nc.vector.tensor_tensor(sf[:], sc[:], cs[:], OP.mult)
                sn = pscr.tile([128, R], bf16, tag="sn")
                nc.vector.tensor_tensor(sn[:], sf[:], st["i2n"][:], OP.mult)
                U = prow.tile([128, 5 * R], bf16, tag=f"U{t}")
                nc.vector.tensor_tensor(
                    sb(U, 0, [[R, 3], [1, R]]),
                    st["tv"][:].rearrange("p (c r) -> p c r", c=3),
                    sb(sn, 0, [[0, 3], [1, R]]), OP.mult)
                nc.vector.tensor_copy(U[:, 3 * R:5 * R], U[:, 0:2 * R])
                st["A"], st["U"] = A, U

            def pair_a(t):
                st = state[t]
                A, U, tt = st["A"], st["U"], st["tt"]

                def A_at(row_off):
                    return sb(A, row_off, [[0, 3], [1, C]])

                def U_at(comp_rot, row_off):
                    return sb(U, comp_rot * R + row_off, [[R, 3], [1, C]])

                def TT_at(row_off):
                    return sb(tt, row_off, [[R, 3], [1, C]])

                cmC = lambda tl: sb(tl, 0, [[C, 3], [1, C]])

                # ----- translation part -----
                if "g1" in st:
                    g1 = st["g1"]
                else:
                    g1 = pscr.tile([128, 5 * C], bf16, tag="g10")
                    nc.vector.tensor_tensor(cmC(g1), TT_at(1), TT_at(0),
                                            OP.subtract)
                    nc.vector.tensor_copy(g1[:, 3 * C:5 * C], g1[:, 0:2 * C])

                def G1(comp_rot):
                    return sb(g1, comp_rot * C, [[C, 3], [1, C]])

                p1 = pscr.tile([128, 3 * C], bf16, tag="p1")
                nc.vector.tensor_tensor(cmC(p1), U_at(1, 0), G1(2), OP.mult)
                p2 = pscr.tile([128, 3 * C], bf16, tag="p2")
                nc.vector.tensor_tensor(cmC(p2), U_at(2, 0), G1(1), OP.mult)
                b = pscr.tile([128, 5 * C], bf16, tag="b")
                nc.vector.tensor_tensor(cmC(b), p1[:], p2[:], OP.subtract)
                nc.vector.tensor_copy(b[:, 3 * C:5 * C], b[:, 0:2 * C])

                def B(comp_rot):
                    return sb(b, comp_rot * C, [[C, 3], [1, C]])

                q1 = pscr.tile([128, 3 * C], bf16, tag="p1")
                nc.vector.tensor_tensor(cmC(q1), U_at(1, 0), B(2), OP.mult)
                q2 = pscr.tile([128, 3 * C], bf16, tag="p2")
                nc.vector.tensor_tensor(cmC(q2), U_at(2, 0), B(1), OP.mult)
                cp = pscr.tile([128, 3 * C], bf16, tag="cp")
                nc.vector.tensor_tensor(cmC(cp), q1[:], q2[:], OP.subtract)
                m = pscr.tile([128, 3 * C], bf16, tag="m")
                nc.vector.tensor_tensor(cmC(m), A_at(0), B(0), OP.mult)

                g = pscr.tile([128, 3 * C], bf16, tag="gg")
                nc.vector.tensor_tensor(g[:], g1[:, 0:3 * C], st["gtt"][:],
                                        OP.subtract)
                gc = pscr.tile([128, 3 * C], bf16, tag="gc")
                nc.vector.tensor_tensor(gc[:], g[:], cp[:], OP.add)
                dfft = pscr.tile([128, 3 * C], bf16, tag="dfft")
                nc.vector.tensor_tensor(dfft[:], gc[:], m[:], OP.subtract)
                st["dfft"] = dfft

                # ----- rotation part: qV = A0*U1 - A1*U0 - U0 x U1 (= 2*qv) --
                mA = pscr.tile([128, C], bf16, tag="mA")
                nc.vector.tensor_tensor(mA[:], A[:, 0:C], A[:, 1:1 + C], OP.mult)
                mU = pscr.tile([128, 3 * C], bf16, tag="p1")
                nc.vector.tensor_tensor(cmC(mU), U_at(0, 0), U_at(0, 1), OP.mult)
                s1 = pscr.tile([128, C], bf16, tag="s1")
                nc.vector.tensor_tensor(s1[:], mU[:, 0:C], mU[:, C:2 * C], OP.add)
                s2 = pscr.tile([128, C], bf16, tag="s2")
                nc.vector.tensor_tensor(s2[:], s1[:], mU[:, 2 * C:3 * C], OP.add)
                qs2 = pscr.tile([128, C], bf16, tag="qs2")
                nc.vector.tensor_tensor(qs2[:], s2[:], mA[:], OP.add)

                pA = pscr.tile([128, 3 * C], bf16, tag="p1")
                nc.vector.tensor_tensor(cmC(pA), A_at(0), U_at(0, 1), OP.mult)
                pB = pscr.tile([128, 3 * C], bf16, tag="p2")
                nc.vector.tensor_tensor(cmC(pB), A_at(1), U_at(0, 0), OP.mult)
                w1 = pscr.tile([128, 3 * C], bf16, tag="w1")
                nc.vector.tensor_tensor(w1[:], pA[:], pB[:], OP.subtract)
                c1 = pscr.tile([128, 3 * C], bf16, tag="p1")
                nc.vector.tensor_tensor(cmC(c1), U_at(1, 0), U_at(2, 1), OP.mult)
                c2 = pscr.tile([128, 3 * C], bf16, tag="p2")
                nc.vector.tensor_tensor(cmC(c2), U_at(2, 0), U_at(1, 1), OP.mult)
                cr = pscr.tile([128, 3 * C], bf16, tag="cr")
                nc.vector.tensor_tensor(cr[:], c1[:], c2[:], OP.subtract)
                qV = pscr.tile([128, 3 * C], bf16, tag=f"qV{t}")
                nc.vector.tensor_tensor(qV[:], w1[:], cr[:], OP.subtract)

                qVsq = pscr.tile([128, 3 * C], bf16, tag="p1")
                mkact(t, None, qVsq[:], qV[:], AF.Square)
                dump_t = pscr.tile([128, 3 * R], bf16, tag="sq")
                acc_t = pacc.tile([128, 1], f32, tag="acct")
                mkact(t, None, dump_t[:, 0:3 * C], st["dfft"][:], AF.Abs,
                      accum_out=acc_t[:])
                acc_ts.append(acc_t)
                nva = pscr.tile([128, C], bf16, tag="s1")
                nc.vector.tensor_tensor(nva[:], qVsq[:, 0:C], qVsq[:, C:2 * C],
                                        OP.add)
                nv2 = pscr.tile([128, C], bf16, tag="s2")
                nc.vector.tensor_tensor(nv2[:], nva[:], qVsq[:, 2 * C:3 * C],
                                        OP.add)

                lq = pscr.tile([128, C], f32, tag="lq")
                mkact(t, 'pairLE', lq[:], nv2[:], AF.Ln, bias=1e-16)
                rs = pscr.tile([128, C], bf16, tag=f"rs{t}")
                mkact(t, 'pairLE', rs[:], lq[:], AF.Exp, scale=-0.5)
                r2 = pscr.tile([128, C], bf16, tag=f"r2{t}")
                nc.vector.tensor_tensor(r2[:], qs2[:], rs[:], OP.mult)
                st["qV"], st["rs"], st["r2"] = qV, rs, r2

            def pair_b(t):
                st = state.pop(t)
                at = pscr.tile([128, C], f32, tag="at")
                mkact(t, 'pairTR', at[:], st["r2"][:], AF.Arctan, scale=-1.0)
                ratio = pscr.tile([128, C], bf16, tag="ratio")
                nc.vector.scalar_tensor_tensor(ratio[:], at[:], PI2, st["rs"][:],
                                               OP.add, OP.mult)
                ld = pscr.tile([128, 3 * C], bf16, tag="w1")
                nc.vector.tensor_tensor(
                    sb(ld, 0, [[C, 3], [1, C]]),
                    sb(st["qV"], 0, [[C, 3], [1, C]]),
                    sb(ratio, 0, [[0, 3], [1, C]]), OP.mult)
                ldiff = pscr.tile([128, 3 * C], bf16, tag="cr")
                nc.vector.tensor_tensor(ldiff[:], ld[:], st["gtq"][:], OP.subtract)
                dump_q = pscr.tile([128, 3 * R], bf16, tag="sq")
                acc_q = pacc.tile([128, 1], f32, tag="accq")
                mkact(t, None, dump_q[:, 0:3 * C], ldiff[:], AF.Abs,
                      accum_out=acc_q[:])
                acc_qs.append(acc_q)

            for t in range(NT):
                dma_phase(t)
            row_n2(0)
            row_lut(0)
            row_trig(0)
            row_n2(1)
            row_n2(2)
            early_g1(0)
            early_g1(1)
            row_lut(1)
            row_lut(2)
            row_trig(1)
            row_trig(2)
            for t in range(NT):
                row_fin(t)
            for t in range(NT):
                pair_a(t)
                pair_b(t)

            # Chain LUT activations so same-table-set groups run contiguously
            # across tiles: 4 table loads total (rowLE, rowTR, pairLE, pairTR).
            order = [('rowLE', 0), ('rowTR', 0),
                     ('rowLE', 1), ('rowLE', 2),
                     ('rowTR', 1), ('rowTR', 2)]
            for ti in range(NT):
                order.append(('pairLE', ti))
                order.append(('pairTR', ti))
            seq = []
            for gname, ti in order:
                seq.extend(groups.get((ti, gname), []))
            for i in range(1, len(seq)):
                _add_dep_helper(seq[i].ins, seq[i - 1].ins, False,
                                "act table-set grouping")

            tot = pacc.tile([128, 2], f32, tag="tot")
            tmp_t = pacc.tile([128, 1], f32, tag="tmpt")
            nc.vector.tensor_tensor(tmp_t[:], acc_ts[0][:], acc_ts[1][:], OP.add)
            nc.vector.tensor_tensor(tot[:, 0:1], tmp_t[:], acc_ts[2][:], OP.add)
            tmp_q = pacc.tile([128, 1], f32, tag="tmpq")
            nc.vector.tensor_tensor(tmp_q[:], acc_qs[0][:], acc_qs[1][:], OP.add)
            nc.vector.tensor_tensor(tot[:, 1:2], tmp_q[:], acc_qs[2][:], OP.add)
            nc.sync.dma_start(out_h[:], tot[:])

    nc.compile()
    return nc


def _get_nc():
    if "nc" not in _BUILT:
        _BUILT["nc"] = _build()
    return _BUILT["nc"]


def run_device(pred, targ, trace=False):
    """pred: (1,T,6) f32, targ: (1,T-1,6) f32 -> (sum|dt|, sum|dq|, exec_ns)"""
    from concourse.bass_utils import run_bass_kernel_spmd

    nc = _get_nc()
    p = np.asarray(pred, dtype=np.float32).reshape(-1, 6)
    g = np.asarray(targ, dtype=np.float32).reshape(-1, 6)
    n_dup = ROWS_PAD - p.shape[0]
    p_pad = np.concatenate([p, np.repeat(p[-1:], n_dup, axis=0)], axis=0)
    g_pad = np.concatenate(
        [g, np.zeros((PAIRS_PAD - g.shape[0], 6), np.float32)], axis=0)

    in_maps = []
    for c in range(N_CORES):
        s = c * PPC
        in_maps.append({
            "pred": np.ascontiguousarray(p_pad[s:s + PPC + 1].T)
                     .astype(ml_dtypes.bfloat16).reshape(-1),
            "targ": np.ascontiguousarray(g_pad[s:s + PPC].T)
                     .astype(ml_dtypes.bfloat16).reshape(-1),
        })
    res = run_bass_kernel_spmd(nc, in_maps, core_ids=list(range(N_CORES)),
                               trace=trace)
    psum = np.stack([res.results[i]["out"] for i in range(N_CORES)])
    st = float(psum[:, :, 0].sum(dtype=np.float64))
    sq = float(psum[:, :, 1].sum(dtype=np.float64))
    return st, sq, res.exec_time_ns


def kernel(pred, targ, srx, srq):
    trace = bool(int(os.environ.get("VO_KERNEL_TRACE", "0")))
    st, sq, _ = run_device(pred, targ, trace=trace)
    t_loss = st / (3.0 * NPAIRS)
    q_loss = sq / (3.0 * NPAIRS)
    srx_v = float(np.asarray(srx).reshape(-1)[0])
    srq_v = float(np.asarray(srq).reshape(-1)[0])
    out = (np.exp(-srx_v) * t_loss + srx_v +
           np.exp(-srq_v) * q_loss + srq_v)
    return np.array([out], dtype=np.float32)
